# revision 38
# baseline (speedup 1.0000x reference)
"""GAT layer (dense-softmax graph attention) on Trainium2, 8 NeuronCores.

Math (matches the reference exactly):
    Wh    = x @ W
    s_src = Wh @ a[:F_OUT] = x @ (W @ a[:F_OUT])
    s_dst = Wh @ a[F_OUT:] = x @ (W @ a[F_OUT:])
    e_ij  = leaky_relu(s_src[i] + s_dst[j], 0.2)
    att   = softmax_row(where(adj != 0, e, 0))
    out   = (att @ Wh).reshape(N, H, F_OUT/H).mean(axis=1)
          = att @ (x @ W_headmean)            # mean commutes with att @ .

Key identities used on device:
    p_ij = exp(adj_ij * lrelu(s_src_i + s_dst_j))   (non-edge -> exp(0) = 1,
           exactly the dense-softmax behaviour of the reference)
    row numerator+denominator in one matmul via a ones column:
           [h'_i | d_i] = sum_j p_ij * [Whm_j | 1]
    out_i = h'_i / d_i

Sharding: 1D partition of output rows i across 8 cores. Each core reads its
transposed row-slice of adj (layout [j, i]: j on partitions, i on the free
dim) plus all of x (needed for the row-global s_dst / Whm), and writes its
own 1024 output rows. No cross-core communication.

Host-side prep (weight folding + layout marshalling only):
    B   = [W @ a_dst | W.reshape(F_IN,H,FM).mean(1)]   [F_IN, 65]
    wsv = W @ a_src                                    [F_IN, 1]
    xT  = x.T (shared across cores), xsT = x[i_slice].T (per core)
    adjc = adj[i_slice, :].T (per core)
"""

import numpy as np

import concourse.bacc as bacc
import concourse.tile as tile
from concourse import mybir
from concourse.bass_utils import run_bass_kernel_spmd
from concourse.masks import make_identity

P = 128
F_IN = 512
F_OUT = 256
HEADS = 4
FM = F_OUT // HEADS        # 64 folded (head-averaged) features
FC = FM + 1                # 65 columns of B: [wd | Wm]
YTC = FM + 2               # 66 columns of a Y chunk: [s_dst | Whm | ones]
KC = F_IN // P             # 4 contraction chunks
N_CORES = 8
N_FULL = 8192
LRELU_SLOPE = 0.2


def build_nc(n=N_FULL, r=None, debug=False, use_gather=True):
    """Build the SPMD Bass program (same program on every core).

    n: total number of graph nodes; r: output rows per core.
    """
    if r is None:
        r = n // N_CORES
    assert n % P == 0 and r % P == 0
    jt_n = n // P              # number of 128-row j-chunks
    ibw = min(512, n)          # xT block width for the Y precompute
    nib = n // ibw
    jcb = ibw // P             # y-chunks per block
    ab = jcb                   # adj j-tiles per DMA batch (== block)
    n_ab = jt_n // ab
    mov = min(r, 512)          # moving free-dim per matmul (fp32 limit 512)
    mh = r // mov
    ich = r // P               # output row chunks
    f32 = mybir.dt.float32
    f32r = mybir.dt.float32r
    i32 = mybir.dt.int32
    AF = mybir.ActivationFunctionType
    OP = mybir.AluOpType

    nc = bacc.Bacc(None, target_bir_lowering=False)
    if not use_gather:
        xT_d = nc.dram_tensor(
            "xT", [P, n // ibw, KC, ibw], f32r, kind="ExternalInput")
    xsT_d = nc.dram_tensor("xsT", [P, KC, r], f32r, kind="ExternalInput")
    adj_d = nc.dram_tensor("adjc", [P, jt_n // ab, ab, r], i32, kind="ExternalInput")
    B_d = nc.dram_tensor("B", [F_IN, FC], f32r, kind="ExternalInput")
    ws_d = nc.dram_tensor("wsv", [F_IN, 1], f32, kind="ExternalInput")
    h_d = nc.dram_tensor("h", [r, FM], f32, kind="ExternalOutput")
    if debug:
        dbg_ssrc = nc.dram_tensor("dbg_ssrc", [P, r], f32, kind="ExternalOutput")
        dbg_y0 = nc.dram_tensor("dbg_y0", [P, YTC], f32, kind="ExternalOutput")
        dbg_y1 = nc.dram_tensor("dbg_y1", [P, YTC], f32, kind="ExternalOutput")
        dbg_u0 = nc.dram_tensor("dbg_u0", [P, r], f32, kind="ExternalOutput")
        dbg_p0 = nc.dram_tensor("dbg_p0", [P, r], f32, kind="ExternalOutput")
        dbg_acc = nc.dram_tensor("dbg_acc", [FM + 1, r], f32, kind="ExternalOutput")

    with tile.TileContext(nc) as tc:
        with (
            tc.tile_pool(name="consts", bufs=1) as consts,
            tc.tile_pool(name="ypool", bufs=jt_n) as ypool,
            tc.tile_pool(name="xpool", bufs=2) as xpool,
            tc.tile_pool(name="adjpool", bufs=4) as adjpool,
            tc.tile_pool(name="upool", bufs=3) as upool,
            tc.tile_pool(name="tpool", bufs=3) as tpool,
            tc.tile_pool(name="ppool", bufs=3) as ppool,
            tc.tile_pool(name="mpool", bufs=2) as mpool,
            tc.tile_pool(name="yps", bufs=2, space="PSUM") as yps,
            tc.tile_pool(name="sps", bufs=1, space="PSUM") as sps,
            tc.tile_pool(name="accps", bufs=1, space="PSUM") as accps,
            tc.tile_pool(name="tailps", bufs=2, space="PSUM") as tailps,
            tc.tile_pool(name="dpool", bufs=1, space="DRAM") as dpool,
        ):
            # ---- constants ----
            b_sb = consts.tile([P, KC, FC], f32r)
            nc.scalar.dma_start(b_sb[:], B_d.rearrange("(kc p) f -> p kc f", p=P))
            ws_sb = consts.tile([P, KC], f32)
            nc.scalar.dma_start(ws_sb[:], ws_d.rearrange("(kc p) o -> p (kc o)", p=P))
            ident = consts.tile([P, P], f32)
            make_identity(nc, ident)

            # ---- s_src broadcast [P, r]: ones(P) outer s_src(i_slice) ----
            # stationary wsb[k, m] = ws[k] for every m, so the matmul output
            # row m is s_src for all partitions m simultaneously. Emitted
            # from the driver loop after block 0 so its 2MB xsT DMA doesn't
            # head-block the first xT block on the scalar ring.
            s_src = consts.tile([P, r], f32)

            def emit_s_src():
                xst = consts.tile([P, KC, r], f32r)
                nc.scalar.dma_start(xst[:], xsT_d[:])
                wsb = consts.tile([P, KC, P], f32r)
                for kc in range(KC):
                    nc.vector.tensor_copy(
                        wsb[:, kc, :], ws_sb[:, kc:kc + 1].to_broadcast([P, P])
                    )
                ssb_ps = sps.tile([P, r], f32)
                for kc in range(KC):
                    for hh in range(mh):
                        nc.tensor.matmul(
                            ssb_ps[:, hh * mov:(hh + 1) * mov],
                            wsb[:, kc, :],
                            xst[:, kc, hh * mov:(hh + 1) * mov],
                            start=(kc == 0),
                            stop=(kc == KC - 1),
                        )
                nc.vector.tensor_copy(s_src[:], ssb_ps[:])
                if not use_gather:
                    return None
                # own rows' Yt = B.T @ xsT, shared with all cores via
                # AllGather so nobody re-reads the full x.
                ybounce = consts.tile([FC, r], f32)
                for h2 in range(r // ibw):
                    yt_ps = yps.tile([FC, ibw], f32, tag="yps")
                    for kc in range(KC):
                        nc.tensor.matmul(
                            yt_ps[:],
                            b_sb[:, kc, :],
                            xst[:, kc, h2 * ibw:(h2 + 1) * ibw],
                            start=(kc == 0),
                            stop=(kc == KC - 1),
                        )
                    nc.vector.tensor_copy(
                        ybounce[:, h2 * ibw:(h2 + 1) * ibw], yt_ps[:])
                own_yt = dpool.tile([FC, r], f32)
                nc.gpsimd.dma_start(own_yt[:], ybounce[:])
                gath = dpool.tile([N_CORES, FC, r], f32, addr_space="Shared")
                nc.gpsimd.collective_compute(
                    "AllGather",
                    OP.bypass,
                    replica_groups=[list(range(N_CORES))],
                    ins=[own_yt.opt()],
                    outs=[gath.opt()],
                )
                return gath

            # ---- stage A: Y chunk production for one 512-row block ----
            # Yt = B.T @ xT-block, computed wide (N=512) so the PE streams
            # at full rate; PE-transposed back to [row-chunk, 66] layout.
            # Row 65 of the transpose input is pinned to 1.0 (affine_select)
            # so each chunk tile comes out as [s_dst | Whm | ones] in one
            # cast-copy: col 0 = s_dst bias, cols 1:66 = the fp32r
            # stationary [Whm | ones] of the accumulation matmul.
            ytiles = []

            def stage_a_block(ib):
                ytb = xpool.tile([P, ibw], f32, tag="ytb")
                nc.gpsimd.memset(ytb[FM:P, :], 0.0)
                if use_gather:
                    bpc = r // ibw      # blocks per core
                    nc.scalar.dma_start(
                        ytb[0:FC, :],
                        gath[ib // bpc, :, (ib % bpc) * ibw:(ib % bpc + 1) * ibw],
                    )
                else:
                    xt = xpool.tile([P, KC, ibw], f32r, tag="xt")
                    nc.scalar.dma_start(xt[:], xT_d[:, ib])
                    yt_ps = yps.tile([FC, ibw], f32, tag="yps")
                    for kc in range(KC):
                        nc.tensor.matmul(
                            yt_ps[:],
                            b_sb[:, kc, :],
                            xt[:, kc, :],
                            start=(kc == 0),
                            stop=(kc == KC - 1),
                        )
                    nc.vector.tensor_copy(ytb[0:FC, :], yt_ps[:])
                for jl in range(jcb):
                    tp = tailps.tile([P, P], f32, tag="tp")
                    nc.tensor.transpose(
                        tp[:], ytb[:, jl * P:(jl + 1) * P], ident[:]
                    )
                    yt = ypool.tile([P, YTC], f32r, tag="yt")
                    nc.vector.tensor_copy(yt[:, 0:FC], tp[:, 0:FC])
                    nc.vector.tensor_scalar(
                        out=yt[:, FC:YTC], in0=tp[:, 0:1],
                        scalar1=0.0, scalar2=1.0,
                        op0=OP.mult, op1=OP.add,
                    )
                    ytiles.append(yt)

            # ---- stage B: one adj batch (ab j-tiles) of the attention ----
            acc = accps.tile([FM + 1, r], f32)
            adjts = {}

            def stage_b_batch(b):
                adjt = adjts.pop(b)
                # j-tiles are processed in pairs: both u tiles of a pair
                # live in one [P, 2, r] tile so a single double-width Exp
                # covers them (halves the ACT per-instruction overhead).
                for fp in range(ab // 2):
                    upair = upool.tile([P, 2, r], f32, tag="u")
                    for h2 in range(2):
                        f = fp * 2 + h2
                        jt = b * ab + f
                        yt = ytiles[jt]
                        sdst_ap = yt[:, 0:1].bitcast(f32)
                        # 3 of 5 tiles on the ACT-heavy split, 2 of 5 on
                        # the DVE-heavy split (measured engine balance).
                        if (jt % 4) != 3:
                            t = tpool.tile([P, r], f32, tag="t")
                            nc.scalar.activation(
                                t[:], s_src[:], AF.Prelu,
                                bias=sdst_ap, scale=1.0, alpha=LRELU_SLOPE,
                            )
                            nc.vector.scalar_tensor_tensor(
                                out=upair[:, h2, :], in0=t[:], scalar=1.0,
                                in1=adjt[:, f, :], op0=OP.mult, op1=OP.mult,
                            )
                        else:
                            zu = tpool.tile([P, r], f32, tag="t")
                            nc.vector.scalar_tensor_tensor(
                                out=zu[:], in0=s_src[:], scalar=sdst_ap,
                                in1=adjt[:, f, :], op0=OP.add, op1=OP.mult,
                            )
                            nc.vector.scalar_tensor_tensor(
                                out=upair[:, h2, :], in0=zu[:],
                                scalar=LRELU_SLOPE, in1=zu[:],
                                op0=OP.mult, op1=OP.max,
                            )
                    ppair = ppool.tile([P, 2, r], f32r, tag="p")
                    nc.scalar.activation(ppair[:], upair[:], AF.Exp)
                    if debug and b == 0 and fp == 0:
                        nc.gpsimd.dma_start(dbg_u0[:], upair[:, 0, :])
                        nc.gpsimd.dma_start(dbg_p0[:], ppair[:, 0, :].bitcast(f32))
                    for h2 in range(2):
                        jt = b * ab + fp * 2 + h2
                        yt = ytiles[jt]
                        for hh in range(mh):
                            nc.tensor.matmul(
                                acc[:, hh * mov:(hh + 1) * mov],
                                yt[:, 1:YTC],
                                ppair[:, h2, hh * mov:(hh + 1) * mov],
                                start=(jt == 0),
                                stop=(jt == jt_n - 1),
                            )

            # ---- fused pipeline: stage A block b overlaps stage B on the
            # chunks produced by block b-1 (keeps every engine's program-
            # order queue alternating between the two stages, so neither
            # stage head-blocks the other on a sequencer).
            if use_gather:
                gath = emit_s_src()
            for b in range(n_ab + 1):
                if b < n_ab:
                    adjt = adjpool.tile([P, ab, r], i32, tag="adj")
                    nc.sync.dma_start(adjt[:], adj_d[:, b])
                    adjts[b] = adjt
                    stage_a_block(b)
                if b == 0 and not use_gather:
                    emit_s_src()
                if b >= 1:
                    stage_b_batch(b - 1)

            if debug:
                nc.gpsimd.dma_start(dbg_ssrc[:], s_src[:])
                nc.gpsimd.dma_start(dbg_y0[:], ytiles[0][:].bitcast(f32))
                nc.gpsimd.dma_start(dbg_y1[:], ytiles[1][:].bitcast(f32))

            # ---- tail: transpose [65, r] -> [r, 65], divide, store ----
            acc_sb = consts.tile([P, r], f32)
            nc.gpsimd.memset(acc_sb[FM:P, :], 0.0)
            nc.vector.tensor_copy(acc_sb[0:FM + 1, :], acc[:])
            if debug:
                nc.gpsimd.dma_start(dbg_acc[:], acc_sb[0:FM + 1, :])
            out_sb = consts.tile([P, ich, FM], f32)
            for ic in range(ich):
                tp = tailps.tile([P, P], f32, tag="tp")
                nc.tensor.transpose(
                    tp[:], acc_sb[:, ic * P:(ic + 1) * P], ident[:]
                )
                rec = mpool.tile([P, 1], f32, tag="rec")
                nc.vector.reciprocal(rec[:], tp[:, FM:FM + 1])
                nc.vector.tensor_scalar_mul(out_sb[:, ic, :], tp[:, 0:FM], rec[:])
            nc.sync.dma_start(h_d.rearrange("(c p) f -> p c f", p=P), out_sb[:])

    return nc


def fold_weights(W, a):
    """Host-side weight folding: B = [W@a_dst | head-mean(W)], ws = W@a_src."""
    W = np.asarray(W, dtype=np.float32)
    a = np.asarray(a, dtype=np.float32).reshape(2 * F_OUT)
    ws = W @ a[:F_OUT]                                   # [F_IN]
    wd = W @ a[F_OUT:]                                   # [F_IN]
    Wm = W.reshape(F_IN, HEADS, FM).mean(axis=1)         # [F_IN, FM]
    B = np.ascontiguousarray(
        np.concatenate([wd[:, None], Wm], axis=1), dtype=np.float32
    )
    return B, np.ascontiguousarray(ws[:, None], dtype=np.float32)


def shard_inputs(x, adj, W, a, n_cores=N_CORES, use_gather=True):
    """Build the per-core input maps."""
    x = np.asarray(x, dtype=np.float32)
    adj = np.ascontiguousarray(np.asarray(adj), dtype=np.int32)
    n = x.shape[0]
    r = n // n_cores
    B, wsv = fold_weights(W, a)
    ibw = min(512, n)
    # pre-swizzle to the exact SBUF tile layouts so every DMA moves one
    # contiguous multi-KB chunk per partition (fast HWDGE descriptor gen)
    # xT tile layout: [p, block, kc, i] = x[block*ibw + i, kc*128 + p]
    xT = None
    if not use_gather:
        xT = np.ascontiguousarray(
            x.reshape(n // ibw, ibw, KC, P).transpose(3, 0, 2, 1))
    in_maps = []
    for c in range(n_cores):
        i0 = c * r
        xs = x[i0:i0 + r]                                # [r, F_IN]
        xsT = np.ascontiguousarray(xs.reshape(r, KC, P).transpose(2, 1, 0))
        # device layout is [j (partitions), i (free)] and the attention
        # mask for output row i, summed index j is adj[i, j] -> transpose
        adjT = np.ascontiguousarray(adj[i0:i0 + r, :].T)  # [n, r]
        ab = ibw // P
        adjr = np.ascontiguousarray(
            adjT.reshape(n // ibw, ab, P, r).transpose(2, 0, 1, 3))
        m = {
            "xsT": xsT,
            "adjc": adjr,
            "B": B,
            "wsv": wsv,
        }
        if not use_gather:
            m["xT"] = xT
        in_maps.append(m)
    return in_maps


def run(x, adj, W, a, n=N_FULL, trace=False, use_gather=True):
    nc = build_nc(n=n, use_gather=use_gather)
    if not nc.is_finalized():
        nc.finalize()
    in_maps = shard_inputs(x, adj, W, a, use_gather=use_gather)
    core_ids = list(range(N_CORES))
    res = run_bass_kernel_spmd(nc, in_maps, core_ids, trace=trace)
    h = np.concatenate([res.results[c]["h"] for c in range(N_CORES)], axis=0)
    return h, res


def kernel(x, adj, W, a, heads=HEADS, **_ignored):
    assert int(heads) == HEADS, f"kernel hardcodes heads={HEADS}"
    assert x.shape == (N_FULL, F_IN) and adj.shape == (N_FULL, N_FULL)
    h, _ = run(x, adj, W, a, n=N_FULL, trace=False)
    return h.astype(np.float32)


# revision 39
# speedup vs baseline: 1.1693x; 1.1693x over previous
"""GAT layer (dense-softmax graph attention) on Trainium2, 8 NeuronCores.

Math (matches the reference exactly):
    Wh    = x @ W
    s_src = Wh @ a[:F_OUT] = x @ (W @ a[:F_OUT])
    s_dst = Wh @ a[F_OUT:] = x @ (W @ a[F_OUT:])
    e_ij  = leaky_relu(s_src[i] + s_dst[j], 0.2)
    att   = softmax_row(where(adj != 0, e, 0))
    out   = (att @ Wh).reshape(N, H, F_OUT/H).mean(axis=1)
          = att @ (x @ W_headmean)            # mean commutes with att @ .

Key identities used on device:
    p_ij = exp(adj_ij * lrelu(s_src_i + s_dst_j))   (non-edge -> exp(0) = 1,
           exactly the dense-softmax behaviour of the reference)
    row numerator+denominator in one matmul via a ones column:
           [h'_i | d_i] = sum_j p_ij * [Whm_j | 1]
    out_i = h'_i / d_i

Sharding: 1D partition of output rows i across 8 cores. Each core reads its
transposed row-slice of adj (layout [j, i]: j on partitions, i on the free
dim) plus all of x (needed for the row-global s_dst / Whm), and writes its
own 1024 output rows. No cross-core communication.

Host-side prep (weight folding + layout marshalling only):
    B   = [W @ a_dst | W.reshape(F_IN,H,FM).mean(1)]   [F_IN, 65]
    wsv = W @ a_src                                    [F_IN, 1]
    xT  = x.T (shared across cores), xsT = x[i_slice].T (per core)
    adjc = adj[i_slice, :].T (per core)
"""

import numpy as np

import concourse.bacc as bacc
import concourse.tile as tile
from concourse import mybir
from concourse.bass_utils import run_bass_kernel_spmd
from concourse.masks import make_identity

P = 128
F_IN = 512
F_OUT = 256
HEADS = 4
FM = F_OUT // HEADS        # 64 folded (head-averaged) features
FC = FM + 1                # 65 columns of B: [wd | Wm]
YTC = FM + 2               # 66 columns of a Y chunk: [s_dst | Whm | ones]
KC = F_IN // P             # 4 contraction chunks
N_CORES = 8
N_FULL = 8192
LRELU_SLOPE = 0.2


def build_nc(n=N_FULL, r=None, debug=False, use_gather=False):
    """Build the SPMD Bass program (same program on every core).

    n: total number of graph nodes; r: output rows per core.
    """
    if r is None:
        r = n // N_CORES
    assert n % P == 0 and r % P == 0
    jt_n = n // P              # number of 128-row j-chunks
    ibw = min(512, n)          # xT block width for the Y precompute
    nib = n // ibw
    jcb = ibw // P             # y-chunks per block
    ab = jcb                   # adj j-tiles per DMA batch (== block)
    n_ab = jt_n // ab
    mov = min(r, 512)          # moving free-dim per matmul (fp32 limit 512)
    mh = r // mov
    ich = r // P               # output row chunks
    f32 = mybir.dt.float32
    f32r = mybir.dt.float32r
    i32 = mybir.dt.int32
    AF = mybir.ActivationFunctionType
    OP = mybir.AluOpType

    nc = bacc.Bacc(None, target_bir_lowering=False)
    if not use_gather:
        xT_d = nc.dram_tensor(
            "xT", [P, n // ibw, KC, ibw], f32r, kind="ExternalInput")
    xsT_d = nc.dram_tensor("xsT", [P, KC, r], f32r, kind="ExternalInput")
    adj_d = nc.dram_tensor("adjc", [P, jt_n // ab, ab, r], i32, kind="ExternalInput")
    B_d = nc.dram_tensor("B", [F_IN, FC], f32r, kind="ExternalInput")
    ws_d = nc.dram_tensor("wsv", [F_IN, 1], f32, kind="ExternalInput")
    h_d = nc.dram_tensor("h", [r, FM], f32, kind="ExternalOutput")
    if debug:
        dbg_ssrc = nc.dram_tensor("dbg_ssrc", [P, r], f32, kind="ExternalOutput")
        dbg_y0 = nc.dram_tensor("dbg_y0", [P, YTC], f32, kind="ExternalOutput")
        dbg_y1 = nc.dram_tensor("dbg_y1", [P, YTC], f32, kind="ExternalOutput")
        dbg_u0 = nc.dram_tensor("dbg_u0", [P, r], f32, kind="ExternalOutput")
        dbg_p0 = nc.dram_tensor("dbg_p0", [P, r], f32, kind="ExternalOutput")
        dbg_acc = nc.dram_tensor("dbg_acc", [FM + 1, r], f32, kind="ExternalOutput")

    with tile.TileContext(nc) as tc:
        with (
            tc.tile_pool(name="consts", bufs=1) as consts,
            tc.tile_pool(name="ypool", bufs=jt_n) as ypool,
            tc.tile_pool(name="xpool", bufs=2) as xpool,
            tc.tile_pool(name="adjpool", bufs=4) as adjpool,
            tc.tile_pool(name="upool", bufs=3) as upool,
            tc.tile_pool(name="tpool", bufs=3) as tpool,
            tc.tile_pool(name="ppool", bufs=3) as ppool,
            tc.tile_pool(name="mpool", bufs=2) as mpool,
            tc.tile_pool(name="yps", bufs=2, space="PSUM") as yps,
            tc.tile_pool(name="sps", bufs=1, space="PSUM") as sps,
            tc.tile_pool(name="accps", bufs=1, space="PSUM") as accps,
            tc.tile_pool(name="tailps", bufs=2, space="PSUM") as tailps,
            tc.tile_pool(name="dpool", bufs=1, space="DRAM") as dpool,
        ):
            # ---- constants ----
            b_sb = consts.tile([P, KC, FC], f32r)
            nc.scalar.dma_start(b_sb[:], B_d.rearrange("(kc p) f -> p kc f", p=P))
            ws_sb = consts.tile([P, KC], f32)
            nc.scalar.dma_start(ws_sb[:], ws_d.rearrange("(kc p) o -> p (kc o)", p=P))
            ident = consts.tile([P, P], f32)
            make_identity(nc, ident)

            # ---- s_src broadcast [P, r]: ones(P) outer s_src(i_slice) ----
            # stationary wsb[k, m] = ws[k] for every m, so the matmul output
            # row m is s_src for all partitions m simultaneously. Emitted
            # from the driver loop after block 0 so its 2MB xsT DMA doesn't
            # head-block the first xT block on the scalar ring.
            s_src = consts.tile([P, r], f32)

            def emit_s_src():
                xst = consts.tile([P, KC, r], f32r)
                nc.scalar.dma_start(xst[:], xsT_d[:])
                wsb = consts.tile([P, KC, P], f32r)
                for kc in range(KC):
                    nc.vector.tensor_copy(
                        wsb[:, kc, :], ws_sb[:, kc:kc + 1].to_broadcast([P, P])
                    )
                ssb_ps = sps.tile([P, r], f32)
                for kc in range(KC):
                    for hh in range(mh):
                        nc.tensor.matmul(
                            ssb_ps[:, hh * mov:(hh + 1) * mov],
                            wsb[:, kc, :],
                            xst[:, kc, hh * mov:(hh + 1) * mov],
                            start=(kc == 0),
                            stop=(kc == KC - 1),
                        )
                nc.vector.tensor_copy(s_src[:], ssb_ps[:])
                if not use_gather:
                    return None
                # own rows' Yt = B.T @ xsT, shared with all cores via
                # AllGather so nobody re-reads the full x.
                ybounce = consts.tile([FC, r], f32)
                for h2 in range(r // ibw):
                    yt_ps = yps.tile([FC, ibw], f32, tag="yps")
                    for kc in range(KC):
                        nc.tensor.matmul(
                            yt_ps[:],
                            b_sb[:, kc, :],
                            xst[:, kc, h2 * ibw:(h2 + 1) * ibw],
                            start=(kc == 0),
                            stop=(kc == KC - 1),
                        )
                    nc.vector.tensor_copy(
                        ybounce[:, h2 * ibw:(h2 + 1) * ibw], yt_ps[:])
                own_yt = dpool.tile([FC, r], f32)
                nc.gpsimd.dma_start(own_yt[:], ybounce[:])
                gath = dpool.tile([N_CORES, FC, r], f32, addr_space="Shared")
                nc.gpsimd.collective_compute(
                    "AllGather",
                    OP.bypass,
                    replica_groups=[list(range(N_CORES))],
                    ins=[own_yt.opt()],
                    outs=[gath.opt()],
                )
                return gath

            # ---- stage A: Y chunk production for one 512-row block ----
            # Yt = B.T @ xT-block, computed wide (N=512) so the PE streams
            # at full rate; PE-transposed back to [row-chunk, 66] layout.
            # Row 65 of the transpose input is pinned to 1.0 (affine_select)
            # so each chunk tile comes out as [s_dst | Whm | ones] in one
            # cast-copy: col 0 = s_dst bias, cols 1:66 = the fp32r
            # stationary [Whm | ones] of the accumulation matmul.
            ytiles = []

            def stage_a_block(ib):
                ytb = xpool.tile([P, ibw], f32, tag="ytb")
                nc.gpsimd.memset(ytb[FM:P, :], 0.0)
                if use_gather:
                    bpc = r // ibw      # blocks per core
                    nc.scalar.dma_start(
                        ytb[0:FC, :],
                        gath[ib // bpc, :, (ib % bpc) * ibw:(ib % bpc + 1) * ibw],
                    )
                else:
                    xt = xpool.tile([P, KC, ibw], f32r, tag="xt")
                    nc.gpsimd.dma_start(xt[:], xT_d[:, ib])
                    yt_ps = yps.tile([FC, ibw], f32, tag="yps")
                    for kc in range(KC):
                        nc.tensor.matmul(
                            yt_ps[:],
                            b_sb[:, kc, :],
                            xt[:, kc, :],
                            start=(kc == 0),
                            stop=(kc == KC - 1),
                        )
                    nc.vector.tensor_copy(ytb[0:FC, :], yt_ps[:])
                for jl in range(jcb):
                    tp = tailps.tile([P, P], f32, tag="tp")
                    nc.tensor.transpose(
                        tp[:], ytb[:, jl * P:(jl + 1) * P], ident[:]
                    )
                    yt = ypool.tile([P, YTC], f32r, tag="yt")
                    nc.vector.tensor_copy(yt[:, 0:FC], tp[:, 0:FC])
                    nc.vector.tensor_scalar(
                        out=yt[:, FC:YTC], in0=tp[:, 0:1],
                        scalar1=0.0, scalar2=1.0,
                        op0=OP.mult, op1=OP.add,
                    )
                    ytiles.append(yt)

            # ---- stage B: one adj batch (ab j-tiles) of the attention ----
            acc = accps.tile([FM + 1, r], f32)
            adjts = {}

            def stage_b_batch(b):
                adjt = adjts.pop(b)
                # j-tiles are processed in pairs: both u tiles of a pair
                # live in one [P, 2, r] tile so a single double-width Exp
                # covers them (halves the ACT per-instruction overhead).
                for fp in range(ab // 2):
                    upair = upool.tile([P, 2, r], f32, tag="u")
                    for h2 in range(2):
                        f = fp * 2 + h2
                        jt = b * ab + f
                        yt = ytiles[jt]
                        sdst_ap = yt[:, 0:1].bitcast(f32)
                        # 3 of 5 tiles on the ACT-heavy split, 2 of 5 on
                        # the DVE-heavy split (measured engine balance).
                        if (jt % 4) != 3:
                            t = tpool.tile([P, r], f32, tag="t")
                            nc.scalar.activation(
                                t[:], s_src[:], AF.Prelu,
                                bias=sdst_ap, scale=1.0, alpha=LRELU_SLOPE,
                            )
                            nc.vector.scalar_tensor_tensor(
                                out=upair[:, h2, :], in0=t[:], scalar=1.0,
                                in1=adjt[:, f, :], op0=OP.mult, op1=OP.mult,
                            )
                        else:
                            zu = tpool.tile([P, r], f32, tag="t")
                            nc.vector.scalar_tensor_tensor(
                                out=zu[:], in0=s_src[:], scalar=sdst_ap,
                                in1=adjt[:, f, :], op0=OP.add, op1=OP.mult,
                            )
                            nc.vector.scalar_tensor_tensor(
                                out=upair[:, h2, :], in0=zu[:],
                                scalar=LRELU_SLOPE, in1=zu[:],
                                op0=OP.mult, op1=OP.max,
                            )
                    ppair = ppool.tile([P, 2, r], f32r, tag="p")
                    nc.scalar.activation(ppair[:], upair[:], AF.Exp)
                    if debug and b == 0 and fp == 0:
                        nc.gpsimd.dma_start(dbg_u0[:], upair[:, 0, :])
                        nc.gpsimd.dma_start(dbg_p0[:], ppair[:, 0, :].bitcast(f32))
                    for h2 in range(2):
                        jt = b * ab + fp * 2 + h2
                        yt = ytiles[jt]
                        for hh in range(mh):
                            nc.tensor.matmul(
                                acc[:, hh * mov:(hh + 1) * mov],
                                yt[:, 1:YTC],
                                ppair[:, h2, hh * mov:(hh + 1) * mov],
                                start=(jt == 0),
                                stop=(jt == jt_n - 1),
                            )

            # ---- fused pipeline: stage A block b overlaps stage B on the
            # chunks produced by block b-1 (keeps every engine's program-
            # order queue alternating between the two stages, so neither
            # stage head-blocks the other on a sequencer).
            if use_gather:
                gath = emit_s_src()
            for b in range(n_ab + 1):
                if b < n_ab:
                    adjt = adjpool.tile([P, ab, r], i32, tag="adj")
                    nc.sync.dma_start(adjt[:], adj_d[:, b])
                    adjts[b] = adjt
                    stage_a_block(b)
                if b == 0 and not use_gather:
                    emit_s_src()
                if b >= 1:
                    stage_b_batch(b - 1)

            if debug:
                nc.gpsimd.dma_start(dbg_ssrc[:], s_src[:])
                nc.gpsimd.dma_start(dbg_y0[:], ytiles[0][:].bitcast(f32))
                nc.gpsimd.dma_start(dbg_y1[:], ytiles[1][:].bitcast(f32))

            # ---- tail: transpose [65, r] -> [r, 65], divide, store ----
            acc_sb = consts.tile([P, r], f32)
            nc.gpsimd.memset(acc_sb[FM:P, :], 0.0)
            nc.vector.tensor_copy(acc_sb[0:FM + 1, :], acc[:])
            if debug:
                nc.gpsimd.dma_start(dbg_acc[:], acc_sb[0:FM + 1, :])
            out_sb = consts.tile([P, ich, FM], f32)
            for ic in range(ich):
                tp = tailps.tile([P, P], f32, tag="tp")
                nc.tensor.transpose(
                    tp[:], acc_sb[:, ic * P:(ic + 1) * P], ident[:]
                )
                rec = mpool.tile([P, 1], f32, tag="rec")
                nc.vector.reciprocal(rec[:], tp[:, FM:FM + 1])
                nc.vector.tensor_scalar_mul(out_sb[:, ic, :], tp[:, 0:FM], rec[:])
            nc.sync.dma_start(h_d.rearrange("(c p) f -> p c f", p=P), out_sb[:])

    return nc


def fold_weights(W, a):
    """Host-side weight folding: B = [W@a_dst | head-mean(W)], ws = W@a_src."""
    W = np.asarray(W, dtype=np.float32)
    a = np.asarray(a, dtype=np.float32).reshape(2 * F_OUT)
    ws = W @ a[:F_OUT]                                   # [F_IN]
    wd = W @ a[F_OUT:]                                   # [F_IN]
    Wm = W.reshape(F_IN, HEADS, FM).mean(axis=1)         # [F_IN, FM]
    B = np.ascontiguousarray(
        np.concatenate([wd[:, None], Wm], axis=1), dtype=np.float32
    )
    return B, np.ascontiguousarray(ws[:, None], dtype=np.float32)


def shard_inputs(x, adj, W, a, n_cores=N_CORES, use_gather=False):
    """Build the per-core input maps."""
    x = np.asarray(x, dtype=np.float32)
    adj = np.ascontiguousarray(np.asarray(adj), dtype=np.int32)
    n = x.shape[0]
    r = n // n_cores
    B, wsv = fold_weights(W, a)
    ibw = min(512, n)
    # pre-swizzle to the exact SBUF tile layouts so every DMA moves one
    # contiguous multi-KB chunk per partition (fast HWDGE descriptor gen)
    # xT tile layout: [p, block, kc, i] = x[block*ibw + i, kc*128 + p]
    xT = None
    if not use_gather:
        xT = np.ascontiguousarray(
            x.reshape(n // ibw, ibw, KC, P).transpose(3, 0, 2, 1))
    in_maps = []
    for c in range(n_cores):
        i0 = c * r
        xs = x[i0:i0 + r]                                # [r, F_IN]
        xsT = np.ascontiguousarray(xs.reshape(r, KC, P).transpose(2, 1, 0))
        # device layout is [j (partitions), i (free)] and the attention
        # mask for output row i, summed index j is adj[i, j] -> transpose
        adjT = np.ascontiguousarray(adj[i0:i0 + r, :].T)  # [n, r]
        ab = ibw // P
        adjr = np.ascontiguousarray(
            adjT.reshape(n // ibw, ab, P, r).transpose(2, 0, 1, 3))
        m = {
            "xsT": xsT,
            "adjc": adjr,
            "B": B,
            "wsv": wsv,
        }
        if not use_gather:
            m["xT"] = xT
        in_maps.append(m)
    return in_maps


def run(x, adj, W, a, n=N_FULL, trace=False, use_gather=False):
    nc = build_nc(n=n, use_gather=use_gather)
    if not nc.is_finalized():
        nc.finalize()
    in_maps = shard_inputs(x, adj, W, a, use_gather=use_gather)
    core_ids = list(range(N_CORES))
    res = run_bass_kernel_spmd(nc, in_maps, core_ids, trace=trace)
    h = np.concatenate([res.results[c]["h"] for c in range(N_CORES)], axis=0)
    return h, res


def kernel(x, adj, W, a, heads=HEADS, **_ignored):
    assert int(heads) == HEADS, f"kernel hardcodes heads={HEADS}"
    assert x.shape == (N_FULL, F_IN) and adj.shape == (N_FULL, N_FULL)
    h, _ = run(x, adj, W, a, n=N_FULL, trace=False)
    return h.astype(np.float32)


# revision 41
# speedup vs baseline: 1.1975x; 1.0241x over previous
"""GAT layer (dense-softmax graph attention) on Trainium2, 8 NeuronCores.

Math (matches the reference exactly):
    Wh    = x @ W
    s_src = Wh @ a[:F_OUT] = x @ (W @ a[:F_OUT])
    s_dst = Wh @ a[F_OUT:] = x @ (W @ a[F_OUT:])
    e_ij  = leaky_relu(s_src[i] + s_dst[j], 0.2)
    att   = softmax_row(where(adj != 0, e, 0))
    out   = (att @ Wh).reshape(N, H, F_OUT/H).mean(axis=1)
          = att @ (x @ W_headmean)            # mean commutes with att @ .

Key identities used on device:
    p_ij = exp(adj_ij * lrelu(s_src_i + s_dst_j))   (non-edge -> exp(0) = 1,
           exactly the dense-softmax behaviour of the reference)
    row numerator+denominator in one matmul via a ones column:
           [h'_i | d_i] = sum_j p_ij * [Whm_j | 1]
    out_i = h'_i / d_i

Sharding: 1D partition of output rows i across 8 cores. Each core reads its
transposed row-slice of adj (layout [j, i]: j on partitions, i on the free
dim) plus all of x (needed for the row-global s_dst / Whm), and writes its
own 1024 output rows. No cross-core communication.

Host-side prep (weight folding + layout marshalling only):
    B   = [W @ a_dst | W.reshape(F_IN,H,FM).mean(1)]   [F_IN, 65]
    wsv = W @ a_src                                    [F_IN, 1]
    xT  = x.T (shared across cores), xsT = x[i_slice].T (per core)
    adjc = adj[i_slice, :].T (per core)
"""

import numpy as np

import concourse.bacc as bacc
import concourse.tile as tile
from concourse import mybir
from concourse.bass_utils import run_bass_kernel_spmd
from concourse.masks import make_identity

P = 128
F_IN = 512
F_OUT = 256
HEADS = 4
FM = F_OUT // HEADS        # 64 folded (head-averaged) features
FC = FM + 1                # 65 columns of B: [wd | Wm]
YTC = FM + 2               # 66 columns of a Y chunk: [s_dst | Whm | ones]
KC = F_IN // P             # 4 contraction chunks
N_CORES = 8
N_FULL = 8192
LRELU_SLOPE = 0.2


def build_nc(n=N_FULL, r=None, debug=False, use_gather=False):
    """Build the SPMD Bass program (same program on every core).

    n: total number of graph nodes; r: output rows per core.
    """
    if r is None:
        r = n // N_CORES
    assert n % P == 0 and r % P == 0
    jt_n = n // P              # number of 128-row j-chunks
    ibw = min(512, n)          # xT block width for the Y precompute
    nib = n // ibw
    jcb = ibw // P             # y-chunks per block
    ab = jcb                   # adj j-tiles per DMA batch (== block)
    n_ab = jt_n // ab
    mov = min(r, 512)          # moving free-dim per matmul (fp32 limit 512)
    mh = r // mov
    ich = r // P               # output row chunks
    f32 = mybir.dt.float32
    f32r = mybir.dt.float32r
    i32 = mybir.dt.int32
    AF = mybir.ActivationFunctionType
    OP = mybir.AluOpType

    nc = bacc.Bacc(None, target_bir_lowering=False)
    if not use_gather:
        xT_d = nc.dram_tensor(
            "xT", [P, n // ibw, KC, ibw], f32r, kind="ExternalInput")
    xsT_d = nc.dram_tensor("xsT", [P, KC, r], f32r, kind="ExternalInput")
    adj_d = nc.dram_tensor("adjc", [P, jt_n // ab, ab, r], i32, kind="ExternalInput")
    B_d = nc.dram_tensor("B", [F_IN, FC], f32r, kind="ExternalInput")
    ws_d = nc.dram_tensor("wsv", [F_IN, 1], f32, kind="ExternalInput")
    h_d = nc.dram_tensor("h", [r, FM], f32, kind="ExternalOutput")
    if debug:
        dbg_ssrc = nc.dram_tensor("dbg_ssrc", [P, r], f32, kind="ExternalOutput")
        dbg_y0 = nc.dram_tensor("dbg_y0", [P, YTC], f32, kind="ExternalOutput")
        dbg_y1 = nc.dram_tensor("dbg_y1", [P, YTC], f32, kind="ExternalOutput")
        dbg_u0 = nc.dram_tensor("dbg_u0", [P, r], f32, kind="ExternalOutput")
        dbg_p0 = nc.dram_tensor("dbg_p0", [P, r], f32, kind="ExternalOutput")
        dbg_acc = nc.dram_tensor("dbg_acc", [FM + 1, r], f32, kind="ExternalOutput")

    with tile.TileContext(nc) as tc:
        with (
            tc.tile_pool(name="consts", bufs=1) as consts,
            tc.tile_pool(name="ypool", bufs=jt_n) as ypool,
            tc.tile_pool(name="xpool", bufs=2) as xpool,
            tc.tile_pool(name="adjpool", bufs=3) as adjpool,
            tc.tile_pool(name="upool", bufs=3) as upool,
            tc.tile_pool(name="tpool", bufs=4) as tpool,
            tc.tile_pool(name="ppool", bufs=4) as ppool,
            tc.tile_pool(name="mpool", bufs=2) as mpool,
            tc.tile_pool(name="yps", bufs=2, space="PSUM") as yps,
            tc.tile_pool(name="sps", bufs=1, space="PSUM") as sps,
            tc.tile_pool(name="accps", bufs=1, space="PSUM") as accps,
            tc.tile_pool(name="tailps", bufs=2, space="PSUM") as tailps,
            tc.tile_pool(name="dpool", bufs=1, space="DRAM") as dpool,
        ):
            # ---- constants ----
            b_sb = consts.tile([P, KC, FC], f32r)
            nc.scalar.dma_start(b_sb[:], B_d.rearrange("(kc p) f -> p kc f", p=P))
            ws_sb = consts.tile([P, KC], f32)
            nc.scalar.dma_start(ws_sb[:], ws_d.rearrange("(kc p) o -> p (kc o)", p=P))
            ident = consts.tile([P, P], f32)
            make_identity(nc, ident)

            # ---- s_src broadcast [P, r]: ones(P) outer s_src(i_slice) ----
            # stationary wsb[k, m] = ws[k] for every m, so the matmul output
            # row m is s_src for all partitions m simultaneously. Emitted
            # from the driver loop after block 0 so its 2MB xsT DMA doesn't
            # head-block the first xT block on the scalar ring.
            s_src = consts.tile([P, r], f32)

            def emit_s_src():
                xst = consts.tile([P, KC, r], f32r)
                nc.scalar.dma_start(xst[:], xsT_d[:])
                wsb = consts.tile([P, KC, P], f32r)
                for kc in range(KC):
                    nc.vector.tensor_copy(
                        wsb[:, kc, :], ws_sb[:, kc:kc + 1].to_broadcast([P, P])
                    )
                ssb_ps = sps.tile([P, r], f32)
                for kc in range(KC):
                    for hh in range(mh):
                        nc.tensor.matmul(
                            ssb_ps[:, hh * mov:(hh + 1) * mov],
                            wsb[:, kc, :],
                            xst[:, kc, hh * mov:(hh + 1) * mov],
                            start=(kc == 0),
                            stop=(kc == KC - 1),
                        )
                nc.vector.tensor_copy(s_src[:], ssb_ps[:])
                if not use_gather:
                    return None
                # own rows' Yt = B.T @ xsT, shared with all cores via
                # AllGather so nobody re-reads the full x.
                ybounce = consts.tile([FC, r], f32)
                for h2 in range(r // ibw):
                    yt_ps = yps.tile([FC, ibw], f32, tag="yps")
                    for kc in range(KC):
                        nc.tensor.matmul(
                            yt_ps[:],
                            b_sb[:, kc, :],
                            xst[:, kc, h2 * ibw:(h2 + 1) * ibw],
                            start=(kc == 0),
                            stop=(kc == KC - 1),
                        )
                    nc.vector.tensor_copy(
                        ybounce[:, h2 * ibw:(h2 + 1) * ibw], yt_ps[:])
                own_yt = dpool.tile([FC, r], f32)
                nc.gpsimd.dma_start(own_yt[:], ybounce[:])
                gath = dpool.tile([N_CORES, FC, r], f32, addr_space="Shared")
                nc.gpsimd.collective_compute(
                    "AllGather",
                    OP.bypass,
                    replica_groups=[list(range(N_CORES))],
                    ins=[own_yt.opt()],
                    outs=[gath.opt()],
                )
                return gath

            # ---- stage A: Y chunk production for one 512-row block ----
            # Yt = B.T @ xT-block, computed wide (N=512, fp32r) so the PE
            # streams at full rate with the B chunks as the (tiny, reused)
            # stationary, then PE-transposed back to row-chunk layout.
            # Each chunk tile is [s_dst | Whm | ones] fp32r: col 0 = s_dst
            # bias (read back as fp32 via bitcast - same bits), cols 1:66 =
            # the fp32r stationary [Whm | ones] of the accumulation matmul
            # (the ones column doubles as the softmax-denominator row).
            ytiles = []

            def stage_a_block(ib):
                ytb = xpool.tile([P, ibw], f32, tag="ytb")
                nc.gpsimd.memset(ytb[FM:P, :], 0.0)
                if use_gather:
                    bpc = r // ibw      # blocks per core
                    nc.scalar.dma_start(
                        ytb[0:FC, :],
                        gath[ib // bpc, :, (ib % bpc) * ibw:(ib % bpc + 1) * ibw],
                    )
                else:
                    xt = xpool.tile([P, KC, ibw], f32r, tag="xt")
                    nc.gpsimd.dma_start(xt[:], xT_d[:, ib])
                    yt_ps = yps.tile([FC, ibw], f32, tag="yps")
                    for kc in range(KC):
                        nc.tensor.matmul(
                            yt_ps[:],
                            b_sb[:, kc, :],
                            xt[:, kc, :],
                            start=(kc == 0),
                            stop=(kc == KC - 1),
                        )
                    nc.vector.tensor_copy(ytb[0:FC, :], yt_ps[:])
                for jl in range(jcb):
                    tp = tailps.tile([P, P], f32, tag="tp")
                    nc.tensor.transpose(
                        tp[:], ytb[:, jl * P:(jl + 1) * P], ident[:]
                    )
                    yt = ypool.tile([P, YTC], f32r, tag="yt")
                    nc.vector.tensor_copy(yt[:, 0:FC], tp[:, 0:FC])
                    nc.vector.tensor_scalar(
                        out=yt[:, FC:YTC], in0=tp[:, 0:1],
                        scalar1=0.0, scalar2=1.0,
                        op0=OP.mult, op1=OP.add,
                    )
                    ytiles.append(yt)

            # ---- stage B: one adj batch (ab j-tiles) of the attention ----
            acc = accps.tile([FM + 1, r], f32)
            adjts = {}

            def stage_b_batch(b):
                adjt = adjts.pop(b)
                # j-tiles are processed in pairs: both u tiles of a pair
                # live in one [P, 2, r] tile so a single double-width Exp
                # covers them (halves the ACT per-instruction overhead).
                for fp in range(ab // 2):
                    upair = upool.tile([P, 2, r], f32, tag="u")
                    for h2 in range(2):
                        f = fp * 2 + h2
                        jt = b * ab + f
                        yt = ytiles[jt]
                        sdst_ap = yt[:, 0:1].bitcast(f32)
                        # 3 of 5 tiles on the ACT-heavy split, 2 of 5 on
                        # the DVE-heavy split (measured engine balance).
                        if (jt % 4) != 3:
                            t = tpool.tile([P, r], f32, tag="t")
                            nc.scalar.activation(
                                t[:], s_src[:], AF.Prelu,
                                bias=sdst_ap, scale=1.0, alpha=LRELU_SLOPE,
                            )
                            nc.vector.scalar_tensor_tensor(
                                out=upair[:, h2, :], in0=t[:], scalar=1.0,
                                in1=adjt[:, f, :], op0=OP.mult, op1=OP.mult,
                            )
                        else:
                            zu = tpool.tile([P, r], f32, tag="t")
                            nc.vector.scalar_tensor_tensor(
                                out=zu[:], in0=s_src[:], scalar=sdst_ap,
                                in1=adjt[:, f, :], op0=OP.add, op1=OP.mult,
                            )
                            nc.vector.scalar_tensor_tensor(
                                out=upair[:, h2, :], in0=zu[:],
                                scalar=LRELU_SLOPE, in1=zu[:],
                                op0=OP.mult, op1=OP.max,
                            )
                    ppair = ppool.tile([P, 2, r], f32r, tag="p")
                    nc.scalar.activation(ppair[:], upair[:], AF.Exp)
                    if debug and b == 0 and fp == 0:
                        nc.gpsimd.dma_start(dbg_u0[:], upair[:, 0, :])
                        nc.gpsimd.dma_start(dbg_p0[:], ppair[:, 0, :].bitcast(f32))
                    for h2 in range(2):
                        jt = b * ab + fp * 2 + h2
                        yt = ytiles[jt]
                        for hh in range(mh):
                            nc.tensor.matmul(
                                acc[:, hh * mov:(hh + 1) * mov],
                                yt[:, 1:YTC],
                                ppair[:, h2, hh * mov:(hh + 1) * mov],
                                start=(jt == 0),
                                stop=(jt == jt_n - 1),
                            )

            # ---- fused pipeline: stage A block b overlaps stage B on the
            # chunks produced by block b-1 (keeps every engine's program-
            # order queue alternating between the two stages, so neither
            # stage head-blocks the other on a sequencer).
            if use_gather:
                gath = emit_s_src()
            for b in range(n_ab + 1):
                if b < n_ab:
                    adjt = adjpool.tile([P, ab, r], i32, tag="adj")
                    nc.sync.dma_start(adjt[:], adj_d[:, b])
                    adjts[b] = adjt
                    stage_a_block(b)
                if b == 0 and not use_gather:
                    emit_s_src()
                if b >= 1:
                    stage_b_batch(b - 1)

            if debug:
                nc.gpsimd.dma_start(dbg_ssrc[:], s_src[:])
                nc.gpsimd.dma_start(dbg_y0[:], ytiles[0][:].bitcast(f32))
                nc.gpsimd.dma_start(dbg_y1[:], ytiles[1][:].bitcast(f32))

            # ---- tail: transpose [65, r] -> [r, 65], divide, store ----
            acc_sb = consts.tile([P, r], f32)
            nc.gpsimd.memset(acc_sb[FM:P, :], 0.0)
            nc.vector.tensor_copy(acc_sb[0:FM + 1, :], acc[:])
            if debug:
                nc.gpsimd.dma_start(dbg_acc[:], acc_sb[0:FM + 1, :])
            out_sb = consts.tile([P, ich, FM], f32)
            for ic in range(ich):
                tp = tailps.tile([P, P], f32, tag="tp")
                nc.tensor.transpose(
                    tp[:], acc_sb[:, ic * P:(ic + 1) * P], ident[:]
                )
                rec = mpool.tile([P, 1], f32, tag="rec")
                nc.vector.reciprocal(rec[:], tp[:, FM:FM + 1])
                nc.vector.tensor_scalar_mul(out_sb[:, ic, :], tp[:, 0:FM], rec[:])
            nc.sync.dma_start(h_d.rearrange("(c p) f -> p c f", p=P), out_sb[:])

    return nc


def fold_weights(W, a):
    """Host-side weight folding: B = [W@a_dst | head-mean(W)], ws = W@a_src."""
    W = np.asarray(W, dtype=np.float32)
    a = np.asarray(a, dtype=np.float32).reshape(2 * F_OUT)
    ws = W @ a[:F_OUT]                                   # [F_IN]
    wd = W @ a[F_OUT:]                                   # [F_IN]
    Wm = W.reshape(F_IN, HEADS, FM).mean(axis=1)         # [F_IN, FM]
    B = np.ascontiguousarray(
        np.concatenate([wd[:, None], Wm], axis=1), dtype=np.float32
    )
    return B, np.ascontiguousarray(ws[:, None], dtype=np.float32)


def shard_inputs(x, adj, W, a, n_cores=N_CORES, use_gather=False):
    """Build the per-core input maps."""
    x = np.asarray(x, dtype=np.float32)
    adj = np.ascontiguousarray(np.asarray(adj), dtype=np.int32)
    n = x.shape[0]
    r = n // n_cores
    B, wsv = fold_weights(W, a)
    ibw = min(512, n)
    # pre-swizzle to the exact SBUF tile layouts so every DMA moves one
    # contiguous multi-KB chunk per partition (fast HWDGE descriptor gen)
    # xT tile layout: [p, block, kc, i] = x[block*ibw + i, kc*128 + p]
    xT = None
    if not use_gather:
        xT = np.ascontiguousarray(
            x.reshape(n // ibw, ibw, KC, P).transpose(3, 0, 2, 1))
    in_maps = []
    for c in range(n_cores):
        i0 = c * r
        xs = x[i0:i0 + r]                                # [r, F_IN]
        xsT = np.ascontiguousarray(xs.reshape(r, KC, P).transpose(2, 1, 0))
        # device layout is [j (partitions), i (free)] and the attention
        # mask for output row i, summed index j is adj[i, j] -> transpose
        adjT = np.ascontiguousarray(adj[i0:i0 + r, :].T)  # [n, r]
        ab = ibw // P
        adjr = np.ascontiguousarray(
            adjT.reshape(n // ibw, ab, P, r).transpose(2, 0, 1, 3))
        m = {
            "xsT": xsT,
            "adjc": adjr,
            "B": B,
            "wsv": wsv,
        }
        if not use_gather:
            m["xT"] = xT
        in_maps.append(m)
    return in_maps


def run(x, adj, W, a, n=N_FULL, trace=False, use_gather=False):
    nc = build_nc(n=n, use_gather=use_gather)
    if not nc.is_finalized():
        nc.finalize()
    in_maps = shard_inputs(x, adj, W, a, use_gather=use_gather)
    core_ids = list(range(N_CORES))
    res = run_bass_kernel_spmd(nc, in_maps, core_ids, trace=trace)
    h = np.concatenate([res.results[c]["h"] for c in range(N_CORES)], axis=0)
    return h, res


def kernel(x, adj, W, a, heads=HEADS, **_ignored):
    assert int(heads) == HEADS, f"kernel hardcodes heads={HEADS}"
    assert x.shape == (N_FULL, F_IN) and adj.shape == (N_FULL, N_FULL)
    h, _ = run(x, adj, W, a, n=N_FULL, trace=False)
    return h.astype(np.float32)
